# revision 24
# baseline (speedup 1.0000x reference)
"""Trainium2 Bass kernel for nn_BitKHopSampler.

Computes out[b, s, v] = y[b, v] + (1 - 2*y[b, v]) * mag[b, s, v] where
mag[b, s, v] = 1 iff v appears in idx[b, s, :].  Equivalently: broadcast
y[b, :] over samples, then flip each hit position v -> 1 - y[b, v].

Sharding: pure data parallel over the batch dim, 64 batches per core on
8 cores.  Per core (all shapes hardcoded):
  - y      (64, 1024)  fp32
  - idx16  (128, 256)  int16, layout [s, b*4+h], duplicate hops -> -1
  - out    (64*128, 1024) fp32

Device pipeline per batch b:
  PE    : ybc  = broadcast of y[b] to 128 partitions via K=4 bf16 matmul
          (y pre-split into exact bf16 hi/mid/lo + a constant-ones row;
          lhsT [1,1,1,0] reconstructs y exactly, [-1,-1,-1,1] gives 1-y)
  ACT   : copy ybc PSUM -> SBUF out tile
  GPSIMD: local_scatter builds the int16 hit mask from idx16
  DVE   : copy_predicated(out, mask, onemy_psum) applies the flips
  DMA   : out tile -> DRAM (512 KiB per batch)
"""

import numpy as np

import concourse.bacc as bacc
import concourse.bass as bass
import concourse.tile as tile
from concourse import mybir
from concourse.bass_utils import run_bass_kernel_spmd


B, S, V, H = 512, 128, 1024, 4
NCORES = 8
BL = B // NCORES  # 64 batches per core

_nc_cache = None


def _build_bass():
    nc = bacc.Bacc("TRN2", debug=False, enable_asserts=False, num_devices=NCORES)
    y_d = nc.dram_tensor("y", [BL, V], mybir.dt.float32, kind="ExternalInput").ap()
    idx_d = nc.dram_tensor(
        "idx16", [S, BL * H], mybir.dt.int16, kind="ExternalInput"
    ).ap()
    lall_d = nc.dram_tensor(
        "lall", [96, 16 * 128], mybir.dt.bfloat16, kind="ExternalInput"
    ).ap()
    out_d = nc.dram_tensor(
        "out", [BL * S, V], mybir.dt.float32, kind="ExternalOutput"
    ).ap()

    f32 = mybir.dt.float32
    bf16 = mybir.dt.bfloat16
    Op = mybir.AluOpType

    with tile.TileContext(nc) as tc:
        with (
            tc.tile_pool(name="const", bufs=1) as cp,
            tc.tile_pool(name="outp", bufs=6) as outp,
            tc.tile_pool(name="maskp", bufs=6) as maskp,
            tc.tile_pool(name="py", bufs=2, space="PSUM") as pyp,
            tc.tile_pool(name="pm", bufs=2, space="PSUM") as pmp,
            tc.tile_pool(name="dramp", bufs=1, space="DRAM") as dramp,
        ):
            # ---- setup: load inputs ----
            Y = cp.tile([BL, V], f32, tag="Y")
            nc.sync.dma_start(out=Y[:], in_=y_d[:])
            IDX = cp.tile([S, BL * H], mybir.dt.int16, tag="IDX")
            nc.sync.dma_start(out=IDX[:], in_=idx_d[:])

            # ---- exact bf16 3-way split of y: y = hi + mid + lo ----
            # SPLT row b = [hi | mid | lo | ones], each V wide; flattening
            # SPLT gives exactly the DRAM layout YSPL[4*b+j] = split j of
            # batch b, so one contiguous DMA builds YSPL.
            SPLT = cp.tile([BL, 4 * V], bf16, tag="SPLT")
            R1 = cp.tile([BL, V], f32, tag="R1")
            R2 = cp.tile([BL, V], f32, tag="R2")
            nc.vector.tensor_copy(out=SPLT[:, 0:V], in_=Y[:])
            nc.vector.tensor_tensor(
                out=R1[:], in0=Y[:], in1=SPLT[:, 0:V], op=Op.subtract
            )
            nc.vector.tensor_copy(out=SPLT[:, V : 2 * V], in_=R1[:])
            nc.vector.tensor_tensor(
                out=R2[:], in0=R1[:], in1=SPLT[:, V : 2 * V], op=Op.subtract
            )
            nc.vector.tensor_copy(out=SPLT[:, 2 * V : 3 * V], in_=R2[:])
            nc.vector.memset(SPLT[:, 3 * V : 4 * V], 1.0)

            # ---- interleaved split layout ----
            # Engine APs may only start at partition 0/32/64, so each YS tile
            # carries 3 K-windows x 8 batch slots = 24 batches in partitions
            # 0..95: partition 32*w + 4*r + j = split j of batch 24*t+8*w+r,
            # j=3 rows all-ones.  Partition interleaving goes through a DRAM
            # round-trip so every SBUF-side DMA uses plain contiguous APs.
            YSPL = dramp.tile([4 * BL, V], bf16, tag="yspl")
            nc.sync.dma_start(out=YSPL[:], in_=SPLT[:])
            YS = [cp.tile([S, V], bf16, name=f"ys{t}", tag=f"ys{t}") for t in range(3)]
            nc.sync.dma_start(out=YS[0][0:96, :], in_=YSPL[0:96, :])
            nc.sync.dma_start(out=YS[1][0:96, :], in_=YSPL[96:192, :])
            nc.sync.dma_start(out=YS[2][0:64, :], in_=YSPL[192:256, :])

            # ---- matmul weights (host-built constant) ----
            # PE K-windows must start at partition 0/32/64, so contract over
            # a full 32-partition window (8 batches) and use a selector lhsT
            # that zeroes every batch except slot r.  Column block 2*r+kind
            # of LALL holds the selector for slot r; kind 0 picks y (rows
            # 4r..4r+2 = 1), kind 1 gives 1-y (rows = -1, row 4r+3 = +1
            # hits the all-ones partition).  Pattern replicated at all three
            # window bases so lhsT and rhs slices share a base partition.
            LALL = cp.tile([96, 16 * 128], bf16, tag="LALL")
            nc.sync.dma_start(out=LALL[:], in_=lall_d[:])

            # Scatter payload + wait-absorbers: InstISA (local_scatter) only
            # supports a limited number of semaphore waits, so satisfy its
            # cross-engine deps (IDX DMA, ONES init) on the gpsimd engine
            # itself; program order then covers them for every scatter.
            ONES = cp.tile([S, H], mybir.dt.int16, tag="ONES")  # scatter payload
            nc.gpsimd.memset(ONES[:], 1)
            IDXPROBE = cp.tile([S, 2], mybir.dt.int16, tag="IDXPROBE")
            nc.gpsimd.tensor_copy(out=IDXPROBE[:], in_=IDX[:, 0:2])

            # ---- per-batch pipeline ----
            for b in range(BL):
                ys = YS[b // 24]
                m = b % 24
                w, r = m // 8, m % 8
                base = 32 * w
                py = pyp.tile([S, V], f32)
                pm = pmp.tile([S, V], f32)
                for h2 in range(2):
                    sl = slice(h2 * 512, (h2 + 1) * 512)
                    nc.tensor.matmul(
                        out=py[:, sl],
                        lhsT=LALL[base : base + 32, 2 * r * 128 : (2 * r + 1) * 128],
                        rhs=ys[base : base + 32, sl],
                        start=True,
                        stop=True,
                    )
                    nc.tensor.matmul(
                        out=pm[:, sl],
                        lhsT=LALL[
                            base : base + 32, (2 * r + 1) * 128 : (2 * r + 2) * 128
                        ],
                        rhs=ys[base : base + 32, sl],
                        start=True,
                        stop=True,
                    )
                ot = outp.tile([S, V], f32)
                nc.scalar.activation(
                    out=ot[:], in_=py[:], func=mybir.ActivationFunctionType.Copy
                )
                mk = maskp.tile([S, V], mybir.dt.int16)
                nc.gpsimd.local_scatter(
                    out_ap=mk[:],
                    data_ap=ONES[:],
                    idxs_ap=IDX[:, H * b : H * b + H],
                    channels=S,
                    num_elems=V,
                    num_idxs=H,
                )
                nc.vector.copy_predicated(out=ot[:], mask=mk[:], data=pm[:])
                nc.sync.dma_start(out=out_d[b * S : (b + 1) * S, :], in_=ot[:])
    # Bacc.compile(): register alloc, event-sem generation (splits waits
    # beyond the ISA limit), library load insertion for local_scatter, and
    # extended-inst ISA codegen.
    nc.compile()
    return nc


def _get_nc():
    global _nc_cache
    if _nc_cache is None:
        _nc_cache = _build_bass()
    return _nc_cache


def _make_lall():
    import ml_dtypes

    pat = np.zeros((32, 16, 128), np.float32)
    for r in range(8):
        pat[4 * r : 4 * r + 3, 2 * r, :] = 1.0
        pat[4 * r : 4 * r + 3, 2 * r + 1, :] = -1.0
        pat[4 * r + 3, 2 * r + 1, :] = 1.0
    blk = pat.reshape(32, 16 * 128)
    return np.ascontiguousarray(
        np.concatenate([blk, blk, blk], axis=0).astype(ml_dtypes.bfloat16)
    )


def _prep_inputs(y, idx):
    """Slice the full inputs into per-core in_maps (host-side index massaging
    only: dtype narrowing, layout transpose, duplicate-hop sentinel)."""
    y = np.asarray(y, dtype=np.float32)
    ii = np.asarray(idx)
    i16 = ii.astype(np.int16)  # values in [0, 1024)
    # reference uses .set semantics: mark duplicate hops within a row so the
    # scatter writes each position once; local_scatter ignores negatives.
    dup = np.zeros(ii.shape, dtype=bool)
    for j in range(1, H):
        for k in range(j):
            dup[..., j] |= ii[..., j] == ii[..., k]
    i16[dup] = -1
    lall = _make_lall()
    in_maps = []
    for c in range(NCORES):
        sl = slice(c * BL, (c + 1) * BL)
        in_maps.append(
            {
                "y": np.ascontiguousarray(y[sl]),
                "idx16": np.ascontiguousarray(
                    i16[sl].transpose(1, 0, 2).reshape(S, BL * H)
                ),
                "lall": lall,
            }
        )
    return in_maps


def _run(y, idx, **spmd_kwargs):
    nc = _get_nc()
    in_maps = _prep_inputs(y, idx)
    res = run_bass_kernel_spmd(nc, in_maps, core_ids=list(range(NCORES)), **spmd_kwargs)
    out = np.empty((B, S, V), dtype=np.float32)
    for c in range(NCORES):
        out[c * BL : (c + 1) * BL] = res.results[c]["out"].reshape(BL, S, V)
    return out, res


def kernel(a=None, b=None, c=None, y=None, idx=None, **_unused):
    # a, b, c are unused by the reference computation.
    out, _ = _run(y, idx)
    return out


# revision 25
# speedup vs baseline: 1.1863x; 1.1863x over previous
"""Trainium2 Bass kernel for nn_BitKHopSampler.

Computes out[b, s, v] = y[b, v] + (1 - 2*y[b, v]) * mag[b, s, v] where
mag[b, s, v] = 1 iff v appears in idx[b, s, :].  Equivalently: broadcast
y[b, :] over samples, then flip each hit position v -> 1 - y[b, v].

Sharding: pure data parallel over the batch dim, 64 batches per core on
8 cores.  Per core (all shapes hardcoded):
  - y      (64, 1024)  fp32
  - idx16  (128, 256)  int16, layout [s, b*4+h], duplicate hops -> -1
  - out    (64*128, 1024) fp32

Device pipeline per batch b:
  PE    : ybc  = broadcast of y[b] to 128 partitions via K=4 bf16 matmul
          (y pre-split into exact bf16 hi/mid/lo + a constant-ones row;
          lhsT [1,1,1,0] reconstructs y exactly, [-1,-1,-1,1] gives 1-y)
  ACT   : copy ybc PSUM -> SBUF out tile
  GPSIMD: local_scatter builds the int16 hit mask from idx16
  DVE   : copy_predicated(out, mask, onemy_psum) applies the flips
  DMA   : out tile -> DRAM (512 KiB per batch)
"""

import numpy as np

import concourse.bacc as bacc
import concourse.bass as bass
import concourse.tile as tile
from concourse import mybir
from concourse.bass_utils import run_bass_kernel_spmd


B, S, V, H = 512, 128, 1024, 4
NCORES = 8
BL = B // NCORES  # 64 batches per core

_nc_cache = None


def _build_bass():
    nc = bacc.Bacc("TRN2", debug=False, enable_asserts=False, num_devices=NCORES)
    y_d = nc.dram_tensor("y", [BL, V], mybir.dt.float32, kind="ExternalInput").ap()
    idx_d = nc.dram_tensor(
        "idx16", [S, BL * H], mybir.dt.int16, kind="ExternalInput"
    ).ap()
    lall_d = nc.dram_tensor(
        "lall", [96, 16 * 128], mybir.dt.bfloat16, kind="ExternalInput"
    ).ap()
    out_d = nc.dram_tensor(
        "out", [BL * S, V], mybir.dt.float32, kind="ExternalOutput"
    ).ap()

    f32 = mybir.dt.float32
    bf16 = mybir.dt.bfloat16
    Op = mybir.AluOpType

    with tile.TileContext(nc) as tc:
        with (
            tc.tile_pool(name="const", bufs=1) as cp,
            tc.tile_pool(name="outp", bufs=8) as outp,
            tc.tile_pool(name="maskp", bufs=8) as maskp,
            tc.tile_pool(name="ps", bufs=4, space="PSUM") as psp,
            tc.tile_pool(name="dramp", bufs=1, space="DRAM") as dramp,
        ):
            # ---- setup: load inputs ----
            Y = cp.tile([BL, V], f32, tag="Y")
            nc.sync.dma_start(out=Y[:], in_=y_d[:])
            IDX = cp.tile([S, BL * H], mybir.dt.int16, tag="IDX")
            nc.sync.dma_start(out=IDX[:], in_=idx_d[:])

            # ---- exact bf16 3-way split of y: y = hi + mid + lo ----
            # SPLT row b = [hi | mid | lo | ones], each V wide; flattening
            # SPLT gives exactly the DRAM layout YSPL[4*b+j] = split j of
            # batch b, so one contiguous DMA builds YSPL.
            SPLT = cp.tile([BL, 4 * V], bf16, tag="SPLT")
            R1 = cp.tile([BL, V], f32, tag="R1")
            R2 = cp.tile([BL, V], f32, tag="R2")
            nc.vector.tensor_copy(out=SPLT[:, 0:V], in_=Y[:])
            nc.vector.tensor_tensor(
                out=R1[:], in0=Y[:], in1=SPLT[:, 0:V], op=Op.subtract
            )
            nc.vector.tensor_copy(out=SPLT[:, V : 2 * V], in_=R1[:])
            nc.vector.tensor_tensor(
                out=R2[:], in0=R1[:], in1=SPLT[:, V : 2 * V], op=Op.subtract
            )
            nc.vector.tensor_copy(out=SPLT[:, 2 * V : 3 * V], in_=R2[:])
            nc.vector.memset(SPLT[:, 3 * V : 4 * V], 1.0)

            # ---- interleaved split layout ----
            # Engine APs may only start at partition 0/32/64, so each YS tile
            # carries 3 K-windows x 8 batch slots = 24 batches in partitions
            # 0..95: partition 32*w + 4*r + j = split j of batch 24*t+8*w+r,
            # j=3 rows all-ones.  Partition interleaving goes through a DRAM
            # round-trip so every SBUF-side DMA uses plain contiguous APs.
            YSPL = dramp.tile([4 * BL, V], bf16, tag="yspl")
            nc.sync.dma_start(out=YSPL[:], in_=SPLT[:])
            YS = [cp.tile([S, V], bf16, name=f"ys{t}", tag=f"ys{t}") for t in range(3)]
            nc.sync.dma_start(out=YS[0][0:96, :], in_=YSPL[0:96, :])
            nc.sync.dma_start(out=YS[1][0:96, :], in_=YSPL[96:192, :])
            nc.sync.dma_start(out=YS[2][0:64, :], in_=YSPL[192:256, :])

            # ---- matmul weights (host-built constant) ----
            # PE K-windows must start at partition 0/32/64, so contract over
            # a full 32-partition window (8 batches) and use a selector lhsT
            # that zeroes every batch except slot r.  Column block 2*r+kind
            # of LALL holds the selector for slot r; kind 0 picks y (rows
            # 4r..4r+2 = 1), kind 1 gives 1-y (rows = -1, row 4r+3 = +1
            # hits the all-ones partition).  Pattern replicated at all three
            # window bases so lhsT and rhs slices share a base partition.
            LALL = cp.tile([96, 16 * 128], bf16, tag="LALL")
            nc.sync.dma_start(out=LALL[:], in_=lall_d[:])

            # Scatter payload + wait-absorbers: InstISA (local_scatter) only
            # supports a limited number of semaphore waits, so satisfy its
            # cross-engine deps (IDX DMA, ONES init) on the gpsimd engine
            # itself; program order then covers them for every scatter.
            ONES = cp.tile([S, H], mybir.dt.int16, tag="ONES")  # scatter payload
            nc.gpsimd.memset(ONES[:], 1)
            IDXPROBE = cp.tile([S, 2], mybir.dt.int16, tag="IDXPROBE")
            nc.gpsimd.tensor_copy(out=IDXPROBE[:], in_=IDX[:, 0:2])

            # ---- per-batch pipeline ----
            # Per-batch: out[s, v] = |ybc[s, v] - mask[s, v]|.  With mask in
            # {0, 1} and y in [0, 1) this equals y (no hit) or 1-y (hit), so
            # the 1-y broadcast matmuls are unnecessary: PE work halves and
            # each batch needs only 2 PSUM banks -> 4-deep PE pipelining.
            for b in range(BL):
                ys = YS[b // 24]
                m = b % 24
                w, r = m // 8, m % 8
                base = 32 * w
                py = psp.tile([S, V], f32)
                for h2 in range(2):
                    sl = slice(h2 * 512, (h2 + 1) * 512)
                    nc.tensor.matmul(
                        out=py[:, sl],
                        lhsT=LALL[base : base + 32, 2 * r * 128 : (2 * r + 1) * 128],
                        rhs=ys[base : base + 32, sl],
                        start=True,
                        stop=True,
                    )
                mk = maskp.tile([S, V], mybir.dt.int16)
                nc.gpsimd.local_scatter(
                    out_ap=mk[:],
                    data_ap=ONES[:],
                    idxs_ap=IDX[:, H * b : H * b + H],
                    channels=S,
                    num_elems=V,
                    num_idxs=H,
                )
                nc.vector.tensor_tensor(
                    out=py[:], in0=py[:], in1=mk[:], op=Op.subtract
                )
                ot = outp.tile([S, V], f32)
                nc.scalar.activation(
                    out=ot[:], in_=py[:], func=mybir.ActivationFunctionType.Abs
                )
                nc.sync.dma_start(out=out_d[b * S : (b + 1) * S, :], in_=ot[:])
    # Bacc.compile(): register alloc, event-sem generation (splits waits
    # beyond the ISA limit), library load insertion for local_scatter, and
    # extended-inst ISA codegen.
    nc.compile()
    return nc


def _get_nc():
    global _nc_cache
    if _nc_cache is None:
        _nc_cache = _build_bass()
    return _nc_cache


def _make_lall():
    import ml_dtypes

    pat = np.zeros((32, 16, 128), np.float32)
    for r in range(8):
        pat[4 * r : 4 * r + 3, 2 * r, :] = 1.0
        pat[4 * r : 4 * r + 3, 2 * r + 1, :] = -1.0
        pat[4 * r + 3, 2 * r + 1, :] = 1.0
    blk = pat.reshape(32, 16 * 128)
    return np.ascontiguousarray(
        np.concatenate([blk, blk, blk], axis=0).astype(ml_dtypes.bfloat16)
    )


def _prep_inputs(y, idx):
    """Slice the full inputs into per-core in_maps (host-side index massaging
    only: dtype narrowing, layout transpose, duplicate-hop sentinel)."""
    y = np.asarray(y, dtype=np.float32)
    ii = np.asarray(idx)
    i16 = ii.astype(np.int16)  # values in [0, 1024)
    # reference uses .set semantics: mark duplicate hops within a row so the
    # scatter writes each position once; local_scatter ignores negatives.
    dup = np.zeros(ii.shape, dtype=bool)
    for j in range(1, H):
        for k in range(j):
            dup[..., j] |= ii[..., j] == ii[..., k]
    i16[dup] = -1
    lall = _make_lall()
    in_maps = []
    for c in range(NCORES):
        sl = slice(c * BL, (c + 1) * BL)
        in_maps.append(
            {
                "y": np.ascontiguousarray(y[sl]),
                "idx16": np.ascontiguousarray(
                    i16[sl].transpose(1, 0, 2).reshape(S, BL * H)
                ),
                "lall": lall,
            }
        )
    return in_maps


def _run(y, idx, **spmd_kwargs):
    nc = _get_nc()
    in_maps = _prep_inputs(y, idx)
    res = run_bass_kernel_spmd(nc, in_maps, core_ids=list(range(NCORES)), **spmd_kwargs)
    out = np.empty((B, S, V), dtype=np.float32)
    for c in range(NCORES):
        out[c * BL : (c + 1) * BL] = res.results[c]["out"].reshape(BL, S, V)
    return out, res


def kernel(a=None, b=None, c=None, y=None, idx=None, **_unused):
    # a, b, c are unused by the reference computation.
    out, _ = _run(y, idx)
    return out
